# revision 18
# baseline (speedup 1.0000x reference)
"""Trainium2 Bass kernel for nn_AttentionSpace_87729001988510.

Batched channel-attention: 3 depthwise convs (K=7) over L, score = QK^T over
L (contracting L), softmax over channels, out = attn @ V.

Sharding: data-parallel over batch B=8 across the 8 NeuronCores (one batch
element per core). Everything below describes the per-core program.

v4 design (vs v3: no q/k DRAM spill round trips, single x load per consumer,
conv work balanced across DVE / Scalar+Pool / PE):
  - All q/k corner turns are SBUF->SBUF XBAR DMA transposes per conv quarter;
    only E does a small DRAM round trip (for the out-matmul stationary).
  - Convs: k0-3 on PE (diag matmuls, PSUM->nat copies on Pool), k4-7+q on
    DVE (TS 4x + TT 2x) and Scalar(mults)+Pool(adds) streams, v split into
    l-halves so out h0 blocks can start before the conv tail finishes.
  - E is normalized in place (DVE TS by 1/rowsum) right after exp, so out
    PSUM results DMA straight to DRAM with no scale-copy.
  - PE order: k-convs -> S(0..7) (d-half gated) -> out h0 blocks -> out h1.
  - Emission is step-interleaved so every in-order queue (esp. sync, which
    carries all loads/turns/stores) sees work in rough execution order.
"""

import numpy as np

import concourse.bass as bass
import concourse.tile as tile
from concourse import bacc, mybir
from concourse.bass_utils import run_bass_kernel_spmd

B = 8
C = 1024
L = 4096
K = 7
PAD = 3
P = 128

NCC = C // P      # channel chunks (8)
NLC = L // P      # l chunks of 128 (32)
LB = 512          # l block for matmuls
NLB = L // LB     # 8
QW = 1024         # conv quarter width
NQ = L // QW      # quarters per chunk (4)
XQ = QW + 2 * PAD # staged conv-input quarter width (1030)
BW = LB + 2 * PAD # staged PE conv-block input width (518)

INV_SQRT_C = 1.0 / np.sqrt(np.float32(C))

f32 = mybir.dt.float32
f16 = mybir.dt.float16
AF = mybir.ActivationFunctionType
ALU = mybir.AluOpType

# ---- conv job assignment (tunable) ----
K_PE = [0, 1, 2, 3]  # k-chunks on PE (diag matmuls)
# (kind, ci, q0, nq): conv of quarters [q0, q0+nq) of chunk ci
DVE_JOBS = [
    ("q", 0, 0, 4), ("k", 4, 0, 4), ("k", 5, 0, 4), ("q", 2, 0, 4),
    ("q", 4, 0, 4), ("q", 5, 0, 4), ("q", 7, 0, 4),
    ("v", 0, 0, 2), ("v", 1, 0, 2), ("v", 4, 0, 2), ("v", 6, 0, 2),
    ("v", 0, 2, 2), ("v", 1, 2, 2), ("v", 4, 2, 2), ("v", 6, 2, 2),
]
SCP_JOBS = [
    ("k", 6, 0, 4), ("k", 7, 0, 4), ("q", 1, 0, 4), ("q", 3, 0, 4),
    ("q", 6, 0, 4),
    ("v", 2, 0, 2), ("v", 3, 0, 2), ("v", 5, 0, 2), ("v", 7, 0, 2),
    ("v", 2, 2, 2), ("v", 3, 2, 2), ("v", 5, 2, 2), ("v", 7, 2, 2),
]
DVE_PRE_Q = 28  # quarters of DVE_JOBS emitted before the S loop (jobs 0-6)
SCP_PRE_Q = 20  # quarters of SCP_JOBS emitted before the S loop (jobs 0-4)


def _flat_quarters(jobs):
    out = []
    for j, (kind, ci, q0, nqn) in enumerate(jobs):
        for qq in range(q0, q0 + nqn):
            out.append((j, kind, ci, qq))
    return out


def _build():
    nc = bacc.Bacc("TRN2", target_bir_lowering=False, debug=False)

    xp_in = nc.dram_tensor("xp", [C, L + 2 * PAD], f16, kind="ExternalInput").ap()
    xpo_in = nc.dram_tensor("xpo", [C, L + 2 * PAD], f16, kind="ExternalInput").ap()
    wq_in = nc.dram_tensor("wq", [C, K], f32, kind="ExternalInput").ap()
    wk_in = nc.dram_tensor("wk", [C, K], f32, kind="ExternalInput").ap()
    wv_in = nc.dram_tensor("wv", [C, K], f32, kind="ExternalInput").ap()
    dk_in = nc.dram_tensor("dk", [C, K * P], f16, kind="ExternalInput").ap()
    out_dram = nc.dram_tensor("out", [C, L], f32, kind="ExternalOutput").ap()
    a_dram = nc.dram_tensor("a_spill", [C, C], f16).ap()
    qnat_dram = nc.dram_tensor("q_nat_spill", [C, L], f16).ap()

    with tile.TileContext(nc) as tc:
        with (
            tc.tile_pool(name="big", bufs=1) as big,
            tc.tile_pool(name="px_pe", bufs=4) as px_pe,
            tc.tile_pool(name="px_dve", bufs=2) as px_dve,
            tc.tile_pool(name="px_scp", bufs=2) as px_scp,
            tc.tile_pool(name="pacc", bufs=1) as pacc,
            tc.tile_pool(name="ptmp", bufs=1) as ptmp,
            tc.tile_pool(name="sctmp", bufs=3) as sctmp,
            tc.tile_pool(name="spacc", bufs=1) as spacc,
            tc.tile_pool(name="pnat_pe", bufs=2) as pnat_pe,
            tc.tile_pool(name="pnat_dve", bufs=2) as pnat_dve,
            tc.tile_pool(name="pnat_scp", bufs=2) as pnat_scp,
            tc.tile_pool(name="pw", bufs=2) as pw,
            tc.tile_pool(name="pqt", bufs=2) as pqt,
            tc.tile_pool(name="pes", bufs=2) as pes,
            tc.tile_pool(name="pet", bufs=2) as pet,
            tc.tile_pool(name="pob", bufs=3) as pob,
            tc.tile_pool(name="pcv", bufs=4, space="PSUM") as pcv,
            tc.tile_pool(name="pb_ps", bufs=2, space="PSUM") as pb_ps,
        ):
            # residents: kT3[l_lo, lc, d]; vres[d_lo, dj, l]
            kT3 = big.tile([P, NLC, C], f16)
            vres = big.tile([P, NCC, L], f16)
            recip8 = big.tile([P, NCC], f32)
            m8 = big.tile([P, NCC], f32)
            mneg8 = big.tile([P, NCC], f32)
            rs8 = big.tile([P, NCC], f32)

            warm = big.tile([P, 1], f32)
            nc.vector.memset(warm[:], 0.0)
            nc.scalar.activation(warm[:], warm[:], AF.Exp)

            qts = {}  # ci -> qt tile (transposed q chunk)

            def load_qt(ci):
                """Transpose-load one q chunk from its DRAM spill."""
                qt = pqt.tile([P, NLC, P], f16, tag="qt", name=f"qt{ci}")
                nc.sync.dma_start_transpose(
                    qt[:], qnat_dram[ci * P : (ci + 1) * P, :]
                )
                qts[ci] = qt

            # ---------------- helpers ----------------
            def load_w(src, name, ci):
                w = pw.tile([P, K], f32, tag=name)
                nc.sync.dma_start(w[:], src[ci * P : (ci + 1) * P, :])
                return w

            def turn_q(kind, ci, qq, natq):
                """k: corner-turn the quarter into kT3 (SBUF->SBUF XBAR DMA).
                q: spill the quarter to DRAM (transpose-loaded in the S loop,
                which keeps qt buffer claims in consumption order)."""
                if kind == "k":
                    nc.sync.dma_start_transpose(
                        kT3[:, qq * 8 : (qq + 1) * 8, ci * P : (ci + 1) * P],
                        natq[:],
                    )
                else:
                    nc.sync.dma_start(
                        qnat_dram[
                            ci * P : (ci + 1) * P, qq * QW : (qq + 1) * QW
                        ],
                        natq[:],
                    )

            # ---- DVE conv quarter: TS (4x) per tap + TT (2x) adds ----
            def dve_conv_q(w, xq, xqo, dst):
                acc_a = pacc.tile([P, QW], f16, tag="acc_a")
                acc_b = pacc.tile([P, QW], f16, tag="acc_b")
                nc.vector.tensor_scalar_mul(acc_a[:], xq[:, 0:QW], w[:, 0:1])
                cur, oth = acc_a, acc_b
                for j in range(1, K):
                    if j % 2 == 0:
                        src = xq[:, j : j + QW]
                    else:
                        src = xqo[:, j - 1 : j - 1 + QW]
                    tmp = ptmp.tile([P, QW], f16, tag="ttmp")
                    nc.vector.tensor_scalar_mul(tmp[:], src, w[:, j : j + 1])
                    o = dst if j == K - 1 else oth[:]
                    nc.vector.tensor_add(o, tmp[:], cur[:])
                    cur, oth = oth, cur

            # ---- Scalar(mults)+Pool(adds) conv quarter ----
            def scp_conv_q(w, xq, xqo, dst):
                acc_a = spacc.tile([P, QW], f16, tag="sacc_a")
                acc_b = spacc.tile([P, QW], f16, tag="sacc_b")
                nc.scalar.activation(
                    acc_a[:], xq[:, 0:QW], AF.Identity, scale=w[:, 0:1]
                )
                cur, oth = acc_a, acc_b
                for j in range(1, K):
                    tmp = sctmp.tile([P, QW], f16, tag="sttmp")
                    nc.scalar.activation(
                        tmp[:], xq[:, j : j + QW], AF.Identity,
                        scale=w[:, j : j + 1],
                    )
                    o = dst if j == K - 1 else oth[:]
                    nc.gpsimd.tensor_add(o, tmp[:], cur[:])
                    cur, oth = oth, cur

            # ---------------- conv quarter streams ----------------
            class Stream:
                def __init__(self, name, jobs, pool, need_xpo, conv_q, natpool):
                    self.name = name
                    self.q = _flat_quarters(jobs)
                    self.pool = pool
                    self.need_xpo = need_xpo
                    self.conv_q = conv_q
                    self.natpool = natpool
                    self.w = {}
                    self.staged = {}
                    self.pend = []
                    self.pos = 0

                def load_i(self, i):
                    if i >= len(self.q):
                        return
                    j, kind, ci, qq = self.q[i]
                    if j not in self.w:
                        self.w[j] = load_w(W_SRC[kind], "w" + self.name, ci)
                    rows = slice(ci * P, (ci + 1) * P)
                    cols = slice(qq * QW, qq * QW + XQ)
                    xq = self.pool.tile([P, XQ], f16, tag="xq")
                    nc.sync.dma_start(xq[:], xp_in[rows, cols])
                    xqo = None
                    if self.need_xpo:
                        xqo = self.pool.tile([P, XQ], f16, tag="xqo")
                        nc.sync.dma_start(xqo[:], xpo_in[rows, cols])
                    self.staged[i] = (xq, xqo)

                def prime(self):
                    self.load_i(0)
                    self.load_i(1)

                def flush(self, n=0):
                    while len(self.pend) > n:
                        turn_q(*self.pend.pop(0))

                def emit_q(self):
                    i = self.pos
                    self.pos += 1
                    j, kind, ci, qq = self.q[i]
                    self.load_i(i + 2)
                    xq, xqo = self.staged.pop(i)
                    w = self.w[j]
                    if kind == "v":
                        dst = vres[:, ci, qq * QW : (qq + 1) * QW]
                        self.conv_q(w, xq, xqo, dst)
                    else:
                        natq = self.natpool.tile([P, QW], f16, tag="natq")
                        self.conv_q(w, xq, xqo, natq[:])
                        self.pend.append((kind, ci, qq, natq))
                        self.flush(1)
                    # drop weight ref once the job's last quarter is emitted
                    if i + 1 >= len(self.q) or self.q[i + 1][0] != j:
                        self.w.pop(j, None)

            W_SRC = {"q": wq_in, "k": wk_in, "v": wv_in}

            dve = Stream("d", DVE_JOBS, px_dve, True, dve_conv_q, pnat_dve)
            scp = Stream("s", SCP_JOBS, px_scp, False, scp_conv_q, pnat_scp)

            # ---- PE k-conv stream (natural diag matmuls, block loads) ----
            pe_st = {"staged": {}, "dm": {}, "pend": [], "pos": 0}
            PE_NBLK = len(K_PE) * NLB  # flat block index

            def pe_load_dm(j):
                ci = K_PE[j]
                dm = pw.tile([P, K * P], f16, tag="dk")
                nc.sync.dma_start(dm[:], dk_in[ci * P : (ci + 1) * P, :])
                pe_st["dm"][j] = dm

            def pe_load_block(b):
                if b >= PE_NBLK:
                    return
                j, lbl = divmod(b, NLB)
                ci = K_PE[j]
                xb = px_pe.tile([P, BW], f16, tag="xb")
                nc.sync.dma_start(
                    xb[:], xp_in[ci * P : (ci + 1) * P, lbl * LB : lbl * LB + BW]
                )
                pe_st["staged"][b] = xb

            def pe_flush(n=0):
                while len(pe_st["pend"]) > n:
                    turn_q(*pe_st["pend"].pop(0))

            def pe_emit_quarter():
                """Emit one conv quarter (2 LB blocks) of the PE stream."""
                b0 = pe_st["pos"]
                pe_st["pos"] += 2
                j, lbl0 = divmod(b0, NLB)
                ci = K_PE[j]
                qq = lbl0 // 2
                dm = pe_st["dm"][j]
                if lbl0 == 0 and j + 1 < len(K_PE):
                    pe_load_dm(j + 1)
                natq = pnat_pe.tile([P, QW], f16, tag="natq")
                for half in range(2):
                    b = b0 + half
                    xb = pe_st["staged"].pop(b)
                    pe_load_block(b + 2)
                    ps = pcv.tile([P, LB], f32, tag="ps512")
                    for jj in range(K):
                        nc.tensor.matmul(
                            ps[:],
                            dm[:, jj * P : (jj + 1) * P],
                            xb[:, jj : jj + LB],
                            start=(jj == 0),
                            stop=(jj == K - 1),
                        )
                    nc.scalar.copy(natq[:, half * LB : (half + 1) * LB], ps[:])
                pe_st["pend"].append(("k", ci, qq, natq))
                pe_flush(1)
                if b0 + 2 >= PE_NBLK:
                    pe_st["dm"].clear()

            # ---------------- prime + step-interleaved conv phase --------
            pe_load_dm(0)
            pe_load_block(0)
            pe_load_block(1)
            dve.prime()
            scp.prime()

            # steps 0-7: pe 2 quarters + scp 1 quarter + dve 2 quarters
            # steps 8-13: scp 1 + dve 2 (dve pre-S done at step 13)
            # steps 14-19: scp 1
            for step in range(20):
                if step < 8:
                    pe_emit_quarter()
                    pe_emit_quarter()
                scp.emit_q()
                if step < 14 and dve.pos < DVE_PRE_Q:
                    dve.emit_q()
                    dve.emit_q()
            pe_flush(0)

            # prime the first two transposed q chunks
            load_qt(0)
            load_qt(1)

            # ---------------- S loop with interleaved conv tail ----------
            def s_mms(ci, qt, sps):
                for h in range(2):
                    for lc in range(NLC):
                        nc.tensor.matmul(
                            sps[:, h * LB : (h + 1) * LB],
                            qt[:, lc, :],
                            kT3[:, lc, h * LB : (h + 1) * LB],
                            start=(lc == 0),
                            stop=(lc == NLC - 1),
                        )

            for ci in range(NCC):
                qt = qts.pop(ci)
                sps = pb_ps.tile([P, C], f32, tag="sps")
                s_mms(ci, qt, sps)
                if ci + 2 < NCC:
                    load_qt(ci + 2)
                # softmax stats (DVE) + exp (scalar) + normalize (DVE)
                m = m8[:, ci : ci + 1]
                nc.vector.tensor_reduce(m, sps[:], mybir.AxisListType.X, ALU.max)
                mneg = mneg8[:, ci : ci + 1]
                nc.vector.tensor_scalar_mul(mneg, m, -float(INV_SQRT_C))
                Es = pes.tile([P, C], f16, tag="Es")
                nc.scalar.activation(
                    Es[:], sps[:], AF.Exp,
                    scale=float(INV_SQRT_C), bias=mneg,
                    accum_out=rs8[:, ci : ci + 1],
                )
                nc.vector.reciprocal(recip8[:, ci : ci + 1], rs8[:, ci : ci + 1])
                nc.vector.tensor_scalar_mul(Es[:], Es[:], recip8[:, ci : ci + 1])
                nc.scalar.dma_start(a_dram[ci * P : (ci + 1) * P, :], Es[:])
                # paced conv-tail quarters (v halves)
                dve.emit_q()
                dve.emit_q()
                scp.emit_q()
                scp.emit_q()
                if ci == 2:
                    scp.flush(0)
                if ci == 4:
                    dve.flush(0)

            # ---------------- out phase: h0 blocks then h1 --------------
            def load_et(ci):
                et = pet.tile([P, NCC, P], f16, tag="et")
                nc.sync.dma_start_transpose(et[:], a_dram[ci * P : (ci + 1) * P, :])
                return et

            def out_block(et, ci, lb):
                ops = pcv.tile([P, LB], f32, tag="ps512")
                for dj in range(NCC):
                    nc.tensor.matmul(
                        ops[:],
                        et[:, dj, :],
                        vres[:, dj, lb * LB : (lb + 1) * LB],
                        start=(dj == 0),
                        stop=(dj == NCC - 1),
                    )
                ob = pob.tile([P, LB], f32, tag="ob")
                nc.scalar.copy(ob[:], ops[:])
                nc.sync.dma_start(
                    out_dram[ci * P : (ci + 1) * P, lb * LB : (lb + 1) * LB],
                    ob[:],
                )

            def out_half(h):
                ets = {0: load_et(0), 1: load_et(1)}
                for ci in range(NCC):
                    et = ets.pop(ci)
                    if ci + 2 < NCC:
                        ets[ci + 2] = load_et(ci + 2)
                    for lb in range(h * 4, h * 4 + 4):
                        out_block(et, ci, lb)

            out_half(0)
            out_half(1)

    nc.compile()
    return nc


_nc_cache = None


def _get_nc():
    global _nc_cache
    if _nc_cache is None:
        _nc_cache = _build()
    return _nc_cache


def _diag_blocks(w: np.ndarray) -> np.ndarray:
    """w: [C, 1, K] fp32 -> [C, K*P] fp16 where row r, block j has
    diag entry at column j*P + (r % P) equal to w[r, 0, j]."""
    d = np.zeros((C, K * P), np.float16)
    r = np.arange(C)
    for j in range(K):
        d[r, j * P + (r % P)] = w[r, 0, j].astype(np.float16)
    return d


def _in_maps(x, q_w, k_w, v_w):
    x = np.asarray(x, dtype=np.float32)
    xp = np.pad(x, ((0, 0), (0, 0), (PAD, PAD))).astype(np.float16)
    xpo = np.pad(x, ((0, 0), (0, 0), (PAD - 1, PAD + 1))).astype(np.float16)
    wq = np.ascontiguousarray(np.asarray(q_w, dtype=np.float32)[:, 0, :])
    wk = np.ascontiguousarray(np.asarray(k_w, dtype=np.float32)[:, 0, :])
    wv = np.ascontiguousarray(np.asarray(v_w, dtype=np.float32)[:, 0, :])
    dk = _diag_blocks(np.asarray(k_w))
    return [
        {
            "xp": np.ascontiguousarray(xp[b]),
            "xpo": np.ascontiguousarray(xpo[b]),
            "wq": wq,
            "wk": wk,
            "wv": wv,
            "dk": dk,
        }
        for b in range(B)
    ]


def kernel(x, q_w, k_w, v_w):
    nc = _get_nc()
    res = run_bass_kernel_spmd(nc, _in_maps(x, q_w, k_w, v_w), list(range(B)))
    out = np.stack([res.results[b]["out"] for b in range(B)]).astype(np.float32)
    return out


# revision 30
# speedup vs baseline: 1.3143x; 1.3143x over previous
"""Trainium2 Bass kernel for nn_AttentionSpace_87729001988510.

Batched channel-attention: 3 depthwise convs (K=7) over L, score = QK^T over
L (contracting L), softmax over channels, out = attn @ V.

Sharding: data-parallel over batch B=8 across the 8 NeuronCores (one batch
element per core). Everything below describes the per-core program.

v4.2 design:
  - Pool engine does NO tensor compute (measured: pool TT ~2.2us/[128,1024]
    and concurrent pool TTs starve DVE by ~4x).
  - Conv lanes: PE (k0-5, diag matmuls, ~13us/chunk), DVE (TS 4x + in-place
    TT chain), HYB (scalar Identity-scale mults + DVE in-place adds).
  - k corner turns are SBUF->SBUF XBAR DMA transposes into resident kT3;
    q spills to DRAM and is transpose-loaded 2-ahead inside the S loop
    (keeps qt buffer claims in consumption order); E does a DRAM round trip.
  - v convs split into l-halves so out h0 blocks start before the conv tail.
  - E normalized in place after exp; out stored as f16 (host casts to f32).
  - Out blocks run dj-outer over 4 PSUM banks to amortize LDWEIGHTS.
"""

import numpy as np

import concourse.bass as bass
import concourse.tile as tile
from concourse import bacc, mybir
from concourse.bass_utils import run_bass_kernel_spmd

B = 8
C = 1024
L = 4096
K = 7
PAD = 3
P = 128

NCC = C // P      # channel chunks (8)
NLC = L // P      # l chunks of 128 (32)
LB = 512          # l block for matmuls
NLB = L // LB     # 8
QW = 1024         # conv quarter width
HW = 2048         # conv half width
XQ = QW + 2 * PAD # staged conv-input quarter width (1030)
XH = HW + 2 * PAD # staged conv-input half width (2054)
BW = LB + 2 * PAD # staged PE conv-block input width (518)

INV_SQRT_C = 1.0 / np.sqrt(np.float32(C))

f32 = mybir.dt.float32
f16 = mybir.dt.float16
AF = mybir.ActivationFunctionType
ALU = mybir.AluOpType

K_PE = [0, 1, 2, 3, 4, 5]  # k-chunks on PE (diag matmuls)
# DVE stream: (kind, ci, q0, nq) in QUARTER units
DVE_JOBS = [
    ("q", 0, 0, 4), ("k", 6, 0, 4), ("k", 7, 0, 4), ("q", 2, 0, 4),
    ("q", 4, 0, 4),
    ("v", 0, 0, 2), ("v", 1, 0, 2), ("v", 4, 0, 2), ("v", 6, 0, 2),
    ("v", 0, 2, 2), ("v", 1, 2, 2), ("v", 4, 2, 2), ("v", 6, 2, 2),
]
DVE_PRE_Q = 20  # quarters emitted before the S loop (jobs 0-4)
# HYB stream: (kind, ci, h0, nh) in HALF units
HYB_JOBS = [
    ("q", 1, 0, 2), ("q", 3, 0, 2), ("q", 5, 0, 2), ("q", 6, 0, 2),
    ("q", 7, 0, 2),
    ("v", 2, 0, 1), ("v", 3, 0, 1), ("v", 5, 0, 1), ("v", 7, 0, 1),
    ("v", 2, 1, 1), ("v", 3, 1, 1), ("v", 5, 1, 1), ("v", 7, 1, 1),
]
HYB_PRE_H = 10  # halves emitted before the S loop (jobs 0-4)


def _flat_units(jobs):
    out = []
    for j, (kind, ci, u0, nu) in enumerate(jobs):
        for uu in range(u0, u0 + nu):
            out.append((j, kind, ci, uu))
    return out


def _build():
    nc = bacc.Bacc("TRN2", target_bir_lowering=False, debug=False)

    xp_in = nc.dram_tensor("xp", [C, L + 2 * PAD], f16, kind="ExternalInput").ap()
    xpo_in = nc.dram_tensor("xpo", [C, L + 2 * PAD], f16, kind="ExternalInput").ap()
    wq_in = nc.dram_tensor("wq", [C, K], f32, kind="ExternalInput").ap()
    wk_in = nc.dram_tensor("wk", [C, K], f32, kind="ExternalInput").ap()
    wv_in = nc.dram_tensor("wv", [C, K], f32, kind="ExternalInput").ap()
    dk_in = nc.dram_tensor("dk", [C, K * P], f16, kind="ExternalInput").ap()
    out_dram = nc.dram_tensor("out", [C, L], f16, kind="ExternalOutput").ap()
    a_dram = nc.dram_tensor("a_spill", [C, C], f16).ap()
    qnat_dram = nc.dram_tensor("q_nat_spill", [C, L], f16).ap()

    with tile.TileContext(nc) as tc:
        with (
            tc.tile_pool(name="big", bufs=1) as big,
            tc.tile_pool(name="px_pe", bufs=3) as px_pe,
            tc.tile_pool(name="px_dve", bufs=2) as px_dve,
            tc.tile_pool(name="px_hyb", bufs=2) as px_hyb,
            tc.tile_pool(name="ptmp", bufs=2) as ptmp,
            tc.tile_pool(name="sctmp", bufs=3) as sctmp,
            tc.tile_pool(name="pnat_pe", bufs=2) as pnat_pe,
            tc.tile_pool(name="pnat_dve", bufs=2) as pnat_dve,
            tc.tile_pool(name="pnat_hyb", bufs=2) as pnat_hyb,
            tc.tile_pool(name="pw", bufs=2) as pw,
            tc.tile_pool(name="pqt", bufs=2) as pqt,
            tc.tile_pool(name="pes", bufs=2) as pes,
            tc.tile_pool(name="pet", bufs=2) as pet,
            tc.tile_pool(name="pob", bufs=2) as pob,
            tc.tile_pool(name="pcv", bufs=4, space="PSUM") as pcv,
            tc.tile_pool(name="pb_ps", bufs=2, space="PSUM") as pb_ps,
        ):
            # residents: kT3[l_lo, lc, d]; vres[d_lo, dj, l]
            kT3 = big.tile([P, NLC, C], f16)
            vres = big.tile([P, NCC, L], f16)
            recip8 = big.tile([P, NCC], f32)
            m8 = big.tile([P, NCC], f32)
            mneg8 = big.tile([P, NCC], f32)
            rs8 = big.tile([P, NCC], f32)

            warm = big.tile([P, 1], f32)
            nc.vector.memset(warm[:], 0.0)
            nc.scalar.activation(warm[:], warm[:], AF.Exp)

            qts = {}  # ci -> qt tile (transposed q chunk)

            def load_qt(ci):
                qt = pqt.tile([P, NLC, P], f16, tag="qt", name=f"qt{ci}")
                nc.sync.dma_start_transpose(
                    qt[:], qnat_dram[ci * P : (ci + 1) * P, :]
                )
                qts[ci] = qt

            def load_w(src, name, ci):
                w = pw.tile([P, K], f32, tag=name)
                nc.sync.dma_start(w[:], src[ci * P : (ci + 1) * P, :])
                return w

            def turn_q(kind, ci, uu, natu, uw):
                """k: corner-turn into kT3 (SBUF->SBUF XBAR DMA).
                q: spill to DRAM (transpose-loaded in the S loop)."""
                nlc8 = uw // P
                if kind == "k":
                    nc.sync.dma_start_transpose(
                        kT3[:, uu * nlc8 : (uu + 1) * nlc8,
                            ci * P : (ci + 1) * P],
                        natu[:],
                    )
                else:
                    nc.sync.dma_start(
                        qnat_dram[
                            ci * P : (ci + 1) * P, uu * uw : (uu + 1) * uw
                        ],
                        natu[:],
                    )

            # ---- DVE conv quarter: TS (4x) + in-place TT chain (2x) ----
            def dve_conv_u(w, xq, xqo, dst):
                prev = None
                for j in range(K):
                    if j % 2 == 0:
                        src = xq[:, j : j + QW]
                    else:
                        src = xqo[:, j - 1 : j - 1 + QW]
                    tmp = ptmp.tile([P, QW], f16, tag="ttmp")
                    nc.vector.tensor_scalar_mul(tmp[:], src, w[:, j : j + 1])
                    if j > 0:
                        o = dst if j == K - 1 else tmp[:]
                        nc.vector.tensor_add(o, prev[:], tmp[:])
                    prev = tmp

            # ---- HYB conv half: scalar mults, DVE in-place adds ----
            def hyb_conv_u(w, xh, _unused, dst):
                prev = None
                for j in range(K):
                    tmp = sctmp.tile([P, HW], f16, tag="sttmp")
                    nc.scalar.activation(
                        tmp[:], xh[:, j : j + HW], AF.Identity,
                        scale=w[:, j : j + 1],
                    )
                    if j > 0:
                        o = dst if j == K - 1 else tmp[:]
                        nc.vector.tensor_add(o, prev[:], tmp[:])
                    prev = tmp

            # ---------------- conv unit streams ----------------
            W_SRC = {"q": wq_in, "k": wk_in, "v": wv_in}

            class Stream:
                def __init__(self, name, jobs, pool, need_xpo, conv_u,
                             natpool, uw):
                    self.name = name
                    self.q = _flat_units(jobs)
                    self.pool = pool
                    self.need_xpo = need_xpo
                    self.conv_u = conv_u
                    self.natpool = natpool
                    self.uw = uw          # unit width (QW or HW)
                    self.xw = uw + 2 * PAD
                    self.w = {}
                    self.staged = {}
                    self.pend = []
                    self.pos = 0

                def load_i(self, i):
                    if i >= len(self.q):
                        return
                    j, kind, ci, uu = self.q[i]
                    if j not in self.w:
                        self.w[j] = load_w(W_SRC[kind], "w" + self.name, ci)
                    rows = slice(ci * P, (ci + 1) * P)
                    cols = slice(uu * self.uw, uu * self.uw + self.xw)
                    xu = self.pool.tile([P, self.xw], f16, tag="xu")
                    nc.sync.dma_start(xu[:], xp_in[rows, cols])
                    xuo = None
                    if self.need_xpo:
                        xuo = self.pool.tile([P, self.xw], f16, tag="xuo")
                        nc.sync.dma_start(xuo[:], xpo_in[rows, cols])
                    self.staged[i] = (xu, xuo)

                def prime(self):
                    self.load_i(0)
                    self.load_i(1)

                def flush(self, n=0):
                    while len(self.pend) > n:
                        turn_q(*self.pend.pop(0))

                def emit_u(self):
                    if self.pos >= len(self.q):
                        return
                    i = self.pos
                    self.pos += 1
                    j, kind, ci, uu = self.q[i]
                    self.load_i(i + 2)
                    xu, xuo = self.staged.pop(i)
                    w = self.w[j]
                    if kind == "v":
                        dst = vres[:, ci, uu * self.uw : (uu + 1) * self.uw]
                        self.conv_u(w, xu, xuo, dst)
                    else:
                        natu = self.natpool.tile([P, self.uw], f16, tag="natu")
                        self.conv_u(w, xu, xuo, natu[:])
                        turn_q(kind, ci, uu, natu, self.uw)
                    if i + 1 >= len(self.q) or self.q[i + 1][0] != j:
                        self.w.pop(j, None)

            dve = Stream("d", DVE_JOBS, px_dve, True, dve_conv_u,
                         pnat_dve, QW)
            hyb = Stream("h", HYB_JOBS, px_hyb, False, hyb_conv_u,
                         pnat_hyb, HW)

            # ---- PE k-conv stream (natural diag matmuls, block loads) ----
            pe_st = {"staged": {}, "dm": {}, "pend": [], "pos": 0}
            PE_NBLK = len(K_PE) * NLB

            def pe_load_dm(j):
                ci = K_PE[j]
                dm = pw.tile([P, K * P], f16, tag="dk", bufs=1)
                nc.sync.dma_start(dm[:], dk_in[ci * P : (ci + 1) * P, :])
                pe_st["dm"][j] = dm

            def pe_load_block(b):
                if b >= PE_NBLK:
                    return
                j, lbl = divmod(b, NLB)
                ci = K_PE[j]
                xb = px_pe.tile([P, BW], f16, tag="xb")
                nc.sync.dma_start(
                    xb[:], xp_in[ci * P : (ci + 1) * P, lbl * LB : lbl * LB + BW]
                )
                pe_st["staged"][b] = xb

            def pe_flush(n=0):
                while len(pe_st["pend"]) > n:
                    turn_q(*pe_st["pend"].pop(0))

            def pe_emit_quarter():
                b0 = pe_st["pos"]
                pe_st["pos"] += 2
                j, lbl0 = divmod(b0, NLB)
                ci = K_PE[j]
                qq = lbl0 // 2
                dm = pe_st["dm"][j]
                if lbl0 == 0 and j + 1 < len(K_PE):
                    pe_load_dm(j + 1)
                natq = pnat_pe.tile([P, QW], f16, tag="natq")
                for half in range(2):
                    b = b0 + half
                    xb = pe_st["staged"].pop(b)
                    pe_load_block(b + 2)
                    ps = pcv.tile([P, LB], f32, tag="ps512")
                    for jj in range(K):
                        nc.tensor.matmul(
                            ps[:],
                            dm[:, jj * P : (jj + 1) * P],
                            xb[:, jj : jj + LB],
                            start=(jj == 0),
                            stop=(jj == K - 1),
                        )
                    if (b0 // 2) % 2 == 0:
                        nc.scalar.copy(
                            natq[:, half * LB : (half + 1) * LB], ps[:]
                        )
                    else:
                        nc.vector.tensor_copy(
                            natq[:, half * LB : (half + 1) * LB], ps[:]
                        )
                turn_q("k", ci, qq, natq, QW)
                if b0 + 2 >= PE_NBLK:
                    pe_st["dm"].clear()

            # ---------------- prime + step-interleaved conv phase --------
            pe_load_dm(0)
            pe_load_block(0)
            pe_load_block(1)
            dve.prime()
            hyb.prime()

            for step in range(12):
                pe_emit_quarter()
                pe_emit_quarter()
                if dve.pos < DVE_PRE_Q:
                    dve.emit_u()
                    if dve.pos < DVE_PRE_Q:
                        dve.emit_u()
                if step >= 2 and hyb.pos < HYB_PRE_H:
                    hyb.emit_u()

            load_qt(0)
            load_qt(1)

            # ---------------- S loop with interleaved conv tail ----------
            def s_mms(qt, sps):
                for h in range(2):
                    for lc in range(NLC):
                        nc.tensor.matmul(
                            sps[:, h * LB : (h + 1) * LB],
                            qt[:, lc, :],
                            kT3[:, lc, h * LB : (h + 1) * LB],
                            start=(lc == 0),
                            stop=(lc == NLC - 1),
                        )

            for ci in range(NCC):
                qt = qts.pop(ci)
                sps = pb_ps.tile([P, C], f32, tag="sps")
                s_mms(qt, sps)
                if ci + 2 < NCC:
                    load_qt(ci + 2)
                # softmax stats (DVE) + exp (scalar) + normalize (DVE)
                m = m8[:, ci : ci + 1]
                nc.vector.tensor_reduce(m, sps[:], mybir.AxisListType.X, ALU.max)
                mneg = mneg8[:, ci : ci + 1]
                nc.vector.tensor_scalar_mul(mneg, m, -float(INV_SQRT_C))
                Es = pes.tile([P, C], f16, tag="Es")
                nc.scalar.activation(
                    Es[:], sps[:], AF.Exp,
                    scale=float(INV_SQRT_C), bias=mneg,
                    accum_out=rs8[:, ci : ci + 1],
                )
                nc.vector.reciprocal(recip8[:, ci : ci + 1], rs8[:, ci : ci + 1])
                nc.vector.tensor_scalar_mul(Es[:], Es[:], recip8[:, ci : ci + 1])
                nc.scalar.dma_start(a_dram[ci * P : (ci + 1) * P, :], Es[:])
                # paced conv-tail units (v halves/quarters)
                dve.emit_u()
                dve.emit_u()
                dve.emit_u()
                hyb.emit_u()
            while dve.pos < len(dve.q):
                dve.emit_u()
            while hyb.pos < len(hyb.q):
                hyb.emit_u()

            # ---------------- out phase: h0 blocks then h1 --------------
            def load_et(ci):
                et = pet.tile([P, NCC, P], f16, tag="et")
                nc.sync.dma_start_transpose(et[:], a_dram[ci * P : (ci + 1) * P, :])
                return et

            def out_ci(et, ci, h):
                """4 l-blocks, dj-outer over 4 PSUM banks (LDW amortized)."""
                lbs = list(range(h * 4, h * 4 + 4))
                pss = [
                    pcv.tile([P, LB], f32, tag="ps512", name=f"ops{i}")
                    for i in range(len(lbs))
                ]
                for dj in range(NCC):
                    for bi, lb in enumerate(lbs):
                        nc.tensor.matmul(
                            pss[bi][:],
                            et[:, dj, :],
                            vres[:, dj, lb * LB : (lb + 1) * LB],
                            start=(dj == 0),
                            stop=(dj == NCC - 1),
                        )
                for bi, lb in enumerate(lbs):
                    ob = pob.tile([P, LB], f16, tag="ob")
                    nc.scalar.copy(ob[:], pss[bi][:])
                    nc.sync.dma_start(
                        out_dram[ci * P : (ci + 1) * P, lb * LB : (lb + 1) * LB],
                        ob[:],
                    )

            for h in range(2):
                ets = {0: load_et(0), 1: load_et(1)}
                for ci in range(NCC):
                    et = ets.pop(ci)
                    if ci + 2 < NCC:
                        ets[ci + 2] = load_et(ci + 2)
                    out_ci(et, ci, h)

    nc.compile()
    return nc


_nc_cache = None


def _get_nc():
    global _nc_cache
    if _nc_cache is None:
        _nc_cache = _build()
    return _nc_cache


def _diag_blocks(w: np.ndarray) -> np.ndarray:
    """w: [C, 1, K] fp32 -> [C, K*P] fp16 where row r, block j has
    diag entry at column j*P + (r % P) equal to w[r, 0, j]."""
    d = np.zeros((C, K * P), np.float16)
    r = np.arange(C)
    for j in range(K):
        d[r, j * P + (r % P)] = w[r, 0, j].astype(np.float16)
    return d


def _in_maps(x, q_w, k_w, v_w):
    x = np.asarray(x, dtype=np.float32)
    xp = np.pad(x, ((0, 0), (0, 0), (PAD, PAD))).astype(np.float16)
    xpo = np.pad(x, ((0, 0), (0, 0), (PAD - 1, PAD + 1))).astype(np.float16)
    wq = np.ascontiguousarray(np.asarray(q_w, dtype=np.float32)[:, 0, :])
    wk = np.ascontiguousarray(np.asarray(k_w, dtype=np.float32)[:, 0, :])
    wv = np.ascontiguousarray(np.asarray(v_w, dtype=np.float32)[:, 0, :])
    dk = _diag_blocks(np.asarray(k_w))
    return [
        {
            "xp": np.ascontiguousarray(xp[b]),
            "xpo": np.ascontiguousarray(xpo[b]),
            "wq": wq,
            "wk": wk,
            "wv": wv,
            "dk": dk,
        }
        for b in range(B)
    ]


def kernel(x, q_w, k_w, v_w):
    nc = _get_nc()
    res = run_bass_kernel_spmd(nc, _in_maps(x, q_w, k_w, v_w), list(range(B)))
    out = np.stack([res.results[b]["out"] for b in range(B)]).astype(np.float32)
    return out
